# revision 37
# baseline (speedup 1.0000x reference)
"""Trainium2 Bass kernel for nn_Encoder_79259326480738.

Strategy (data-parallel over batch, 8 cores, 1 image each):
  - Everything channel-major [C, N] on-chip; conv1x1 = matmul with
    host-pre-transposed weights as the stationary operand (bf16 operands,
    fp32 PSUM accumulation; stats/residual paths stay fp32).
  - Sync-BN: per-core partial sums (Sum, SumSq) + tiny AllReduces.  The
    qkv-layer stats ride two collectives (qk+x first, v-sumsq second) so
    both overlap the v-conv / early attention.
  - Attention per head: logits computed TRANSPOSED (attT[m,n] = k^T q) so
    that exp(attT) can feed P@V directly; softmax denominator via
    ones-matmul on the PE; no max-subtraction (logits are ~N(0,1)).
  - v produced directly transposed (vT[m, vch]) by swapping the conv's
    stationary/moving operands; BN on v commutes through attention:
    o_bn = s_v * (o_raw * r) + t_v per-partition on the output.
  - mu_v from linearity (w_v @ mu_x); var_v from squared-vT ones-matmul.
  - rsqrt on DVE (Newton + bit-trick seed): ACT only loads exp/gelu sets.
"""

import sys

if "/opt/trn_rl_repo" not in sys.path:
    sys.path.insert(0, "/opt/trn_rl_repo")

import numpy as np

import concourse.bacc as bacc
import concourse.bass as bass
import concourse.mybir as mybir
import concourse.tile as tile
from concourse.bass_utils import run_bass_kernel_spmd

F32 = mybir.dt.float32
BF16 = mybir.dt.bfloat16
U32 = mybir.dt.uint32
AF = mybir.ActivationFunctionType
ALU = mybir.AluOpType
AX = mybir.AxisListType

N_CORES = 8
C = 384          # model channels
N = 1024         # tokens per image (32*32)
HEADS = 8
DK = 32
DV = 128
QKC = 512        # q channels (256) + k channels (256)
VCH = 1024       # v channels total
F1 = 768         # fc1 out channels
TOK = 8192.0     # global tokens (B * N) for batch statistics
EPS = 1e-5
MAGIC = 0x5F3759DF


def _rsqrt(nc, sb, z, k, name):
    """Newton rsqrt on DVE for [128, k] fp32 tile z (values >= EPS)."""
    y = sb.tile([128, k], F32, name=f"{name}_y", tag="stats", bufs=48)
    t1 = sb.tile([128, k], F32, name=f"{name}_t", tag="stats", bufs=48)
    yu = y.bitcast(U32)
    zu = z.bitcast(U32)
    nc.vector.tensor_scalar(yu, zu, 1, None, ALU.logical_shift_right)
    # y_bits = MAGIC - y_bits, done in the fp32 ALU (inputs are < 2^31 so the
    # ~64-ULP fp32 rounding only perturbs the Newton seed negligibly).
    nc.vector.tensor_scalar(yu, yu, -1.0, float(MAGIC), ALU.mult, ALU.add)
    for _ in range(3):
        nc.vector.tensor_tensor(t1, y, y, ALU.mult)
        nc.vector.tensor_tensor(t1, t1, z, ALU.mult)
        nc.vector.tensor_scalar(t1, t1, -0.5, 1.5, ALU.mult, ALU.add)
        nc.vector.tensor_tensor(y, y, t1, ALU.mult)
    return y


def _bn_scale_shift(nc, sb, sum_t, ssq_t, g_t, b_t, k, name, mean_override=None):
    """Given global Sum/SumSq [128,k], produce s = g*rsqrt(var+eps),
    t = b - mean*s.  Returns (s, t)."""
    mean = sb.tile([128, k], F32, name=f"{name}_mean", tag="stats", bufs=48)
    var = sb.tile([128, k], F32, name=f"{name}_var", tag="stats", bufs=48)
    if mean_override is None:
        nc.vector.tensor_scalar(mean, sum_t, 1.0 / TOK, None, ALU.mult)
    else:
        mean = mean_override
    # var + eps = (ssq/TOK + eps) - mean^2
    m2 = sb.tile([128, k], F32, name=f"{name}_m2", tag="stats", bufs=48)
    nc.vector.tensor_tensor(m2, mean, mean, ALU.mult)
    nc.vector.tensor_scalar(var, ssq_t, 1.0 / TOK, EPS, ALU.mult, ALU.add)
    nc.vector.tensor_tensor(var, var, m2, ALU.subtract)
    rsn = _rsqrt(nc, sb, var, k, name)
    s = sb.tile([128, k], F32, name=f"{name}_s", tag="stats", bufs=48)
    t = sb.tile([128, k], F32, name=f"{name}_t2", tag="stats", bufs=48)
    nc.vector.tensor_tensor(s, g_t, rsn, ALU.mult)
    nc.vector.tensor_tensor(t, mean, s, ALU.mult)
    nc.vector.tensor_tensor(t, b_t, t, ALU.subtract)  # t = b - mean*s
    return s, t


def build():
    nc = bacc.Bacc("TRN2", target_bir_lowering=False, debug=False,
                   num_devices=N_CORES)

    xb_d = nc.dram_tensor("xb", [C, N], BF16, kind="ExternalInput")
    wqk_d = nc.dram_tensor("wqk", [C, QKC], BF16, kind="ExternalInput")
    wv_d = nc.dram_tensor("wv", [C, VCH], BF16, kind="ExternalInput")
    wm_d = nc.dram_tensor("wm", [VCH, C], BF16, kind="ExternalInput")
    wf1_d = nc.dram_tensor("wf1", [C, F1], BF16, kind="ExternalInput")
    wf2_d = nc.dram_tensor("wf2", [F1, C], BF16, kind="ExternalInput")
    gqk_d = nc.dram_tensor("gqk", [128, 4], F32, kind="ExternalInput")
    bqk_d = nc.dram_tensor("bqk", [128, 4], F32, kind="ExternalInput")
    gv_d = nc.dram_tensor("gv", [128, 8], F32, kind="ExternalInput")
    bv_d = nc.dram_tensor("bv", [128, 8], F32, kind="ExternalInput")
    gm_d = nc.dram_tensor("gm", [128, 3], F32, kind="ExternalInput")
    bm_d = nc.dram_tensor("bm", [128, 3], F32, kind="ExternalInput")
    gf1_d = nc.dram_tensor("gf1", [128, 6], F32, kind="ExternalInput")
    bf1_d = nc.dram_tensor("bf1", [128, 6], F32, kind="ExternalInput")
    gf2_d = nc.dram_tensor("gf2", [128, 3], F32, kind="ExternalInput")
    bf2_d = nc.dram_tensor("bf2", [128, 3], F32, kind="ExternalInput")
    out_d = nc.dram_tensor("out", [C, N], F32, kind="ExternalOutput")

    RG = [list(range(N_CORES))]

    with tile.TileContext(nc) as tc:
        with (
            tc.tile_pool(name="sb", bufs=1) as sb,
            tc.tile_pool(name="ps", bufs=1, space="PSUM") as ps,
            tc.tile_pool(name="dr", bufs=1, space="DRAM") as dr,
        ):
            # ---------- persistent SBUF tiles ----------
            xb_t = sb.tile([128, 3 * N], BF16, name="xb_t")
            wqk_t = sb.tile([128, 3 * 512], BF16, name="wqk_t")
            wv_t = sb.tile([128, 3 * VCH], BF16, name="wv_t")
            wm_t = sb.tile([128, 8 * C], BF16, name="wm_t")
            wf1_t = sb.tile([128, 3 * F1], BF16, name="wf1_t")
            wf2_t = sb.tile([128, 6 * C], BF16, name="wf2_t")
            # fp32 raw conv outputs: qk_raw then (later) fc1 raw share a slot
            qk_t = sb.tile([128, 4 * N], F32, name="qk_t", tag="k6",
                           padded_shape=[128, 6 * N], bufs=1)
            # bf16 activations: BN'd q/k then (later) gelu(fc1) share a slot
            qkb_t = sb.tile([128, 4 * N], BF16, name="qkb_t", tag="bfk",
                            padded_shape=[128, 6 * N], bufs=1)
            vT_t = sb.tile([128, 8 * N], BF16, name="vT_t")
            go_t = sb.tile([128, 8 * N], BF16, name="go_t")
            x1_t = sb.tile([128, 3 * N], F32, name="x1_t")
            ymb_t = sb.tile([128, 3 * N], BF16, name="ymb_t")
            w1s_t = sb.tile([128, 3 * F1], BF16, name="w1s_t")
            tmb_t = sb.tile([128, 3], BF16, name="tmb_t")
            tw1_t = sb.tile([128, 6], F32, name="tw1_t")

            gqk_t = sb.tile([128, 4], F32, name="gqk_t")
            bqk_t = sb.tile([128, 4], F32, name="bqk_t")
            gv_t = sb.tile([128, 8], F32, name="gv_t")
            bv_t = sb.tile([128, 8], F32, name="bv_t")
            gm_t = sb.tile([128, 3], F32, name="gm_t")
            bm_t = sb.tile([128, 3], F32, name="bm_t")
            gf1_t = sb.tile([128, 6], F32, name="gf1_t")
            bf1_t = sb.tile([128, 6], F32, name="bf1_t")
            gf2_t = sb.tile([128, 3], F32, name="gf2_t")
            bf2_t = sb.tile([128, 3], F32, name="bf2_t")

            ones_cb = sb.tile([128, 1], BF16, name="ones_cb")

            # stats accumulators
            qksum = sb.tile([128, 4], F32, name="qksum")
            qkssq = sb.tile([128, 4], F32, name="qkssq")
            xsum = sb.tile([128, 3], F32, name="xsum")
            msum = sb.tile([128, 3], F32, name="msum")
            mssq = sb.tile([128, 3], F32, name="mssq")
            f1sum = sb.tile([128, 6], F32, name="f1sum")
            f1ssq = sb.tile([128, 6], F32, name="f1ssq")
            f2sum = sb.tile([128, 3], F32, name="f2sum")
            f2ssq = sb.tile([128, 3], F32, name="f2ssq")

            # ---------- input DMAs (first wave: what the convs need) -------
            for cc in range(3):
                nc.sync.dma_start(xb_t[:, cc * N:(cc + 1) * N],
                                  xb_d[cc * 128:(cc + 1) * 128, :])
                nc.sync.dma_start(wqk_t[:, cc * 512:(cc + 1) * 512],
                                  wqk_d[cc * 128:(cc + 1) * 128, :])
            for cc in range(3):
                nc.sync.dma_start(wv_t[:, cc * VCH:(cc + 1) * VCH],
                                  wv_d[cc * 128:(cc + 1) * 128, :])
            for tt, dd in [(gqk_t, gqk_d), (bqk_t, bqk_d), (gv_t, gv_d),
                           (bv_t, bv_d), (gm_t, gm_d), (bm_t, bm_d),
                           (gf1_t, gf1_d), (bf1_t, bf1_d), (gf2_t, gf2_d),
                           (bf2_t, bf2_d)]:
                nc.sync.dma_start(tt[:], dd[:])
            nc.vector.memset(ones_cb[:], 1.0)

            # ---------- x column sums (for mu_v via linearity) ----------
            for cc in range(3):
                nc.vector.reduce_sum(xsum[:, cc:cc + 1],
                                     xb_t[:, cc * N:(cc + 1) * N], axis=AX.X)

            # ---------- qk conv: qk_raw[512, N] ----------
            with nc.named_scope("qkconv"):
                for ot in range(4):
                    cps = ps.tile([128, N], F32, name="cps", tag="big", bufs=2)
                    for cc in range(3):
                        lhs = wqk_t[:, cc * 512 + ot * 128: cc * 512 + ot * 128 + 128]
                        rhs = xb_t[:, cc * N:(cc + 1) * N]
                        nc.tensor.matmul(cps[:, 0:512], lhs, rhs[:, 0:512],
                                         start=(cc == 0), stop=(cc == 2))
                        nc.tensor.matmul(cps[:, 512:1024], lhs, rhs[:, 512:1024],
                                         start=(cc == 0), stop=(cc == 2))
                    nc.vector.tensor_scalar(qk_t[:, ot * N:(ot + 1) * N], cps[:],
                                            1.0, None, ALU.mult, ALU.add,
                                            accum_out=qksum[:, ot:ot + 1])
                    sqs = sb.tile([128, N], BF16, name="sqs", tag="sqs", bufs=2)
                    nc.scalar.activation(sqs[:], cps[:], AF.Square,
                                         accum_out=qkssq[:, ot:ot + 1])

            # ---------- AllReduce #1a (qk BN stats + x sums) ----------
            # pack/unpack DMAs ride the gpsimd software DGE so they are not
            # queued behind the bulk input DMAs on the HW queues
            ar1_i = dr.tile([1408], F32, name="ar1_i")
            ar1_o = dr.tile([1408], F32, name="ar1_o")
            nc.gpsimd.dma_start(ar1_i[0:512].rearrange("(p t) -> p t", p=128), qksum[:])
            nc.gpsimd.dma_start(ar1_i[512:1024].rearrange("(p t) -> p t", p=128), qkssq[:])
            nc.gpsimd.dma_start(ar1_i[1024:1408].rearrange("(p t) -> p t", p=128), xsum[:])
            nc.gpsimd.collective_compute("AllReduce", ALU.add, replica_groups=RG,
                                         ins=[ar1_i.opt()], outs=[ar1_o.opt()])

            # ---------- vT conv: vT[m, vch] (overlaps AllReduce #1a) ----------
            with nc.named_scope("vconv"):
                vss_ps = ps.tile([128, N], F32, name="vss_ps", tag="rs", bufs=1)
                for mt in range(8):
                    cps = ps.tile([128, N], F32, name="cpsv", tag="big", bufs=2)
                    for cc in range(3):
                        lhs = xb_t[:, cc * N + mt * 128: cc * N + mt * 128 + 128]
                        for nh in range(2):
                            nc.tensor.matmul(
                                cps[:, nh * 512:(nh + 1) * 512], lhs,
                                wv_t[:, cc * VCH + nh * 512: cc * VCH + (nh + 1) * 512],
                                start=(cc == 0), stop=(cc == 2))
                    nc.vector.tensor_copy(vT_t[:, mt * N:(mt + 1) * N], cps[:])
                    v2 = sb.tile([128, N], BF16, name="v2", tag="v2", bufs=2)
                    nc.vector.tensor_tensor(v2[:], vT_t[:, mt * N:(mt + 1) * N],
                                            vT_t[:, mt * N:(mt + 1) * N], ALU.mult)
                    for nh in range(2):
                        nc.tensor.matmul(vss_ps[0:1, nh * 512:(nh + 1) * 512],
                                         ones_cb[:], v2[:, nh * 512:(nh + 1) * 512],
                                         start=(mt == 0), stop=(mt == 7))
                vss_row = sb.tile([1, N], F32, name="vss_row", tag="row", bufs=3)
                nc.vector.tensor_copy(vss_row[:], vss_ps[0:1, :])

            # ---------- AllReduce #1b (v sumsq; overlaps attention start) ----
            ar1b_i = dr.tile([1024], F32, name="ar1b_i")
            ar1b_o = dr.tile([1024], F32, name="ar1b_o")
            nc.gpsimd.dma_start(ar1b_i[:].rearrange("(a n) -> a n", a=1), vss_row[:])
            nc.gpsimd.collective_compute("AllReduce", ALU.add, replica_groups=RG,
                                         ins=[ar1b_i.opt()], outs=[ar1b_o.opt()])
            # unpacks (emitted after both triggers so the gpsimd FIFO never
            # blocks a later trigger on an earlier completion)
            qksum_g = sb.tile([128, 4], F32, name="qksum_g")
            qkssq_g = sb.tile([128, 4], F32, name="qkssq_g")
            xsum_g = sb.tile([128, 3], F32, name="xsum_g")
            nc.gpsimd.dma_start(qksum_g[:], ar1_o[0:512].rearrange("(p t) -> p t", p=128))
            nc.gpsimd.dma_start(qkssq_g[:], ar1_o[512:1024].rearrange("(p t) -> p t", p=128))
            nc.gpsimd.dma_start(xsum_g[:], ar1_o[1024:1408].rearrange("(p t) -> p t", p=128))
            vssq_pm = sb.tile([128, 8], F32, name="vssq_pm")
            nc.gpsimd.dma_start(vssq_pm[:], ar1b_o.rearrange("(t p) -> p t", p=128))

            # ---------- second DMA wave: weights needed after attention ----
            for hh in range(8):
                nc.sync.dma_start(wm_t[:, hh * C:(hh + 1) * C],
                                  wm_d[hh * 128:(hh + 1) * 128, :])
            for cc in range(3):
                nc.sync.dma_start(wf1_t[:, cc * F1:(cc + 1) * F1],
                                  wf1_d[cc * 128:(cc + 1) * 128, :])
            for cc in range(6):
                nc.sync.dma_start(wf2_t[:, cc * C:(cc + 1) * C],
                                  wf2_d[cc * 128:(cc + 1) * 128, :])

            # ---------- BN scale/shift for q,k; apply ----------
            s_qk, t_qk = _bn_scale_shift(nc, sb, qksum_g, qkssq_g, gqk_t, bqk_t,
                                         4, "qk")
            for ot in range(4):
                nc.vector.tensor_scalar(qkb_t[:, ot * N:(ot + 1) * N],
                                        qk_t[:, ot * N:(ot + 1) * N],
                                        s_qk[:, ot:ot + 1], t_qk[:, ot:ot + 1],
                                        ALU.mult, ALU.add)

            # mu_v = w_v^T @ mu_x  (linearity of the conv)
            mu_x = sb.tile([128, 3], F32, name="mu_x")
            nc.vector.tensor_scalar(mu_x[:], xsum_g[:], 1.0 / TOK, None, ALU.mult)
            mu_xb = sb.tile([128, 3], BF16, name="mu_xb")
            nc.vector.tensor_copy(mu_xb[:], mu_x[:])
            mv_ps = ps.tile([128, N], F32, name="mv_ps", tag="rs", bufs=1)
            for cc in range(3):
                for nh in range(2):
                    nc.tensor.matmul(
                        mv_ps[0:1, nh * 512:(nh + 1) * 512], mu_xb[:, cc:cc + 1],
                        wv_t[:, cc * VCH + nh * 512: cc * VCH + (nh + 1) * 512],
                        start=(cc == 0), stop=(cc == 2))
            mv_row = sb.tile([1, N], F32, name="mv_row", tag="row", bufs=3)
            nc.vector.tensor_copy(mv_row[:], mv_ps[0:1, :])
            mv_dr = dr.tile([N], F32, name="mv_dr", tag="drow", bufs=4)
            nc.sync.dma_start(mv_dr[:].rearrange("(a n) -> a n", a=1), mv_row[:])
            mu_v = sb.tile([128, 8], F32, name="mu_v")
            nc.sync.dma_start(mu_v[:], mv_dr.rearrange("(t p) -> p t", p=128))
            s_v, t_v = _bn_scale_shift(nc, sb, None, vssq_pm, gv_t, bv_t, 8, "v",
                                       mean_override=mu_v)

            # ---------- attention ----------
            attT = [[] for _ in range(HEADS)]

            def emit_qkT_pair(p):
                h0, h1 = 2 * p, 2 * p + 1
                with nc.named_scope(f"qkT{p}"):
                    for mt in range(8):
                        for h in (h0, h1):
                            j = h % 4
                            strip = slice(32 * j, 32 * j + 32)
                            qcol = (h // 4) * N
                            kcol = (2 + h // 4) * N
                            lg = ps.tile([128, N], F32, name="lg", tag="big", bufs=2)
                            lhs = qkb_t[strip, kcol + mt * 128: kcol + mt * 128 + 128]
                            for nh in range(2):
                                nc.tensor.matmul(
                                    lg[:, nh * 512:(nh + 1) * 512], lhs,
                                    qkb_t[strip, qcol + nh * 512: qcol + (nh + 1) * 512],
                                    start=True, stop=True,
                                    tile_position=(32 * j, 0))
                            at = sb.tile([128, N], BF16, name="at", tag="attexp",
                                         bufs=11)
                            nc.scalar.activation(at[:], lg[:], AF.Exp)
                            attT[h].append(at)

            def emit_post_pair(p):
                h0 = 2 * p
                with nc.named_scope(f"post{p}"):
                    # softmax denominators for both heads via ones-matmul,
                    # gathered into one [2, N] row tile so the
                    # scatter/recip/gather roundtrip is paid once per pair
                    rs_rows = []
                    for i, h in enumerate((h0, h0 + 1)):
                        rs_ps = ps.tile([128, N], F32, name="rs_ps", tag="rs",
                                        bufs=1)
                        for mc in range(8):
                            for nh in range(2):
                                nc.tensor.matmul(
                                    rs_ps[0:1, nh * 512:(nh + 1) * 512],
                                    ones_cb[:],
                                    attT[h][mc][:, nh * 512:(nh + 1) * 512],
                                    start=(mc == 0), stop=(mc == 7))
                        rs_row = sb.tile([1, N], F32, name="rs_row", tag="row",
                                         bufs=3)
                        nc.vector.tensor_copy(rs_row[0:1, :], rs_ps[0:1, :])
                        rs_rows.append(rs_row)
                    # single-hop SBUF->SBUF partition scatter/gather
                    rs_pm = sb.tile([128, 16], F32, name="rs_pm", tag="rspm", bufs=2)
                    for i in range(2):
                        for t in range(8):
                            nc.sync.dma_start(rs_pm[:, i * 8 + t:i * 8 + t + 1],
                                              rs_rows[i][0:1, t * 128:(t + 1) * 128])
                    r_pm = sb.tile([128, 16], F32, name="r_pm", tag="rpm", bufs=2)
                    nc.vector.reciprocal(r_pm[:], rs_pm[:])
                    r_rows = []
                    for i in range(2):
                        r_row = sb.tile([1, N], F32, name="r_row", tag="row",
                                        bufs=3)
                        for t in range(8):
                            nc.sync.dma_start(r_row[0:1, t * 128:(t + 1) * 128],
                                              r_pm[:, i * 8 + t:i * 8 + t + 1])
                        r_rows.append(r_row)
                    for i, h in enumerate((h0, h0 + 1)):
                        # o_raw = vT^T @ attT_exp   [dv, n]
                        o_ps = ps.tile([128, N], F32, name="o_ps", tag="o", bufs=1)
                        for mc in range(8):
                            lhs = vT_t[:, mc * N + h * 128: mc * N + h * 128 + 128]
                            for nh in range(2):
                                nc.tensor.matmul(
                                    o_ps[:, nh * 512:(nh + 1) * 512], lhs,
                                    attT[h][mc][:, nh * 512:(nh + 1) * 512],
                                    start=(mc == 0), stop=(mc == 7))
                        # evacuate immediately (scaled by s_v) so o_ps frees
                        # fast; 1/rowsum lands later in SBUF; gelu deferred
                        o_sc = sb.tile([128, N], F32, name="o_sc", tag="osc",
                                       bufs=2)
                        nc.vector.tensor_scalar(o_sc[:], o_ps[:],
                                                s_v[:, h:h + 1], None, ALU.mult)
                        r_bc = sb.tile([128, N], F32, name="r_bc", tag="rbc",
                                       bufs=2)
                        nc.gpsimd.partition_broadcast(r_bc[:], r_rows[i][0:1, :])
                        nc.vector.tensor_tensor(go_t[:, h * N:(h + 1) * N],
                                                o_sc[:], r_bc[:], ALU.mult)

            def emit_gelu(h):
                # deferred gelu (bias = t_v), in place on the bf16 go tile
                nc.scalar.activation(go_t[:, h * N:(h + 1) * N],
                                     go_t[:, h * N:(h + 1) * N],
                                     AF.Gelu, bias=t_v[:, h:h + 1], scale=1.0)

            emit_qkT_pair(0)
            for p in range(1, 4):
                emit_qkT_pair(p)
                emit_post_pair(p - 1)
            for h in range(4):
                emit_gelu(h)
            emit_post_pair(3)
            for h in range(4, 8):
                emit_gelu(h)

            # ---------- merge conv + BN + residual ----------
            ym_t = sb.tile([128, 3 * N], F32, name="ym_t", tag="c3k", bufs=2)
            with nc.named_scope("mergeconv"):
                for ot in range(3):
                    cps = ps.tile([128, N], F32, name="cpsm", tag="big", bufs=2)
                    for hh in range(8):
                        lhs = wm_t[:, hh * C + ot * 128: hh * C + ot * 128 + 128]
                        rhs = go_t[:, hh * N:(hh + 1) * N]
                        for nh in range(2):
                            nc.tensor.matmul(cps[:, nh * 512:(nh + 1) * 512], lhs,
                                             rhs[:, nh * 512:(nh + 1) * 512],
                                             start=(hh == 0), stop=(hh == 7))
                    nc.vector.tensor_scalar(ym_t[:, ot * N:(ot + 1) * N], cps[:],
                                            1.0, None, ALU.mult, ALU.add,
                                            accum_out=msum[:, ot:ot + 1])
                    sqs = sb.tile([128, N], BF16, name="sqsm", tag="sqs", bufs=2)
                    nc.scalar.activation(sqs[:], cps[:], AF.Square,
                                         accum_out=mssq[:, ot:ot + 1])
                    nc.vector.tensor_copy(ymb_t[:, ot * N:(ot + 1) * N],
                                          ym_t[:, ot * N:(ot + 1) * N])

            ar2_i = dr.tile([768], F32, name="ar2_i")
            ar2_o = dr.tile([768], F32, name="ar2_o")
            nc.gpsimd.dma_start(ar2_i[0:384].rearrange("(p t) -> p t", p=128), msum[:])
            nc.gpsimd.dma_start(ar2_i[384:768].rearrange("(p t) -> p t", p=128), mssq[:])
            nc.gpsimd.collective_compute("AllReduce", ALU.add, replica_groups=RG,
                                         ins=[ar2_i.opt()], outs=[ar2_o.opt()])
            msum_g = sb.tile([128, 3], F32, name="msum_g")
            mssq_g = sb.tile([128, 3], F32, name="mssq_g")
            nc.gpsimd.dma_start(msum_g[:], ar2_o[0:384].rearrange("(p t) -> p t", p=128))
            nc.gpsimd.dma_start(mssq_g[:], ar2_o[384:768].rearrange("(p t) -> p t", p=128))
            s_m, t_m = _bn_scale_shift(nc, sb, msum_g, mssq_g, gm_t, bm_t, 3, "m")
            # fc1 is split as h1 = W1^T x + (W1*s_m)^T ym + (W1^T t_m) x 1 so
            # the x-side matmul can run during AllReduce #2.  Prepare the
            # scaled weight copy and the rank-1 bias now (tiny).
            for cc in range(3):
                nc.vector.tensor_scalar(w1s_t[:, cc * F1:(cc + 1) * F1],
                                        wf1_t[:, cc * F1:(cc + 1) * F1],
                                        s_m[:, cc:cc + 1], None, ALU.mult)
            nc.vector.tensor_copy(tmb_t[:], t_m[:])
            tw1_ps = ps.tile([128, 8], F32, name="tw1_ps", tag="rs", bufs=1)
            for ot in range(6):
                for cc in range(3):
                    nc.tensor.matmul(
                        tw1_ps[:, ot:ot + 1],
                        wf1_t[:, cc * F1 + ot * 128: cc * F1 + ot * 128 + 128],
                        tmb_t[:, cc:cc + 1],
                        start=(cc == 0), stop=(cc == 2))
            nc.vector.tensor_copy(tw1_t[:], tw1_ps[:, 0:6])
            # x1 = (x + t) + ym * s  (final-residual source; off critical path)
            for ot in range(3):
                x1s = x1_t[:, ot * N:(ot + 1) * N]
                nc.vector.tensor_scalar(x1s, xb_t[:, ot * N:(ot + 1) * N],
                                        t_m[:, ot:ot + 1], None, ALU.add)
                nc.vector.scalar_tensor_tensor(x1s,
                                               ym_t[:, ot * N:(ot + 1) * N],
                                               s_m[:, ot:ot + 1], x1s,
                                               ALU.mult, ALU.add)

            # ---------- fc1 + BN + gelu ----------
            h1_t = sb.tile([128, 6 * N], F32, name="h1_t", tag="k6", bufs=1)
            with nc.named_scope("fc1"):
                for ot in range(6):
                    cps = ps.tile([128, N], F32, name="cps1", tag="big", bufs=2)
                    for cc in range(3):
                        lhs = wf1_t[:, cc * F1 + ot * 128: cc * F1 + ot * 128 + 128]
                        rhs = xb_t[:, cc * N:(cc + 1) * N]
                        for nh in range(2):
                            nc.tensor.matmul(cps[:, nh * 512:(nh + 1) * 512], lhs,
                                             rhs[:, nh * 512:(nh + 1) * 512],
                                             start=(cc == 0), stop=False)
                    for cc in range(3):
                        lhs = w1s_t[:, cc * F1 + ot * 128: cc * F1 + ot * 128 + 128]
                        rhs = ymb_t[:, cc * N:(cc + 1) * N]
                        for nh in range(2):
                            nc.tensor.matmul(cps[:, nh * 512:(nh + 1) * 512], lhs,
                                             rhs[:, nh * 512:(nh + 1) * 512],
                                             start=False, stop=(cc == 2))
                    # h1 = psum + t_w1 (per-partition), stats accumulate on out
                    nc.vector.tensor_scalar(h1_t[:, ot * N:(ot + 1) * N], cps[:],
                                            tw1_t[:, ot:ot + 1], None, ALU.add,
                                            ALU.add,
                                            accum_out=f1sum[:, ot:ot + 1])
                    sqs = sb.tile([128, N], BF16, name="sqs1", tag="sqs", bufs=2)
                    nc.scalar.activation(sqs[:], h1_t[:, ot * N:(ot + 1) * N],
                                         AF.Square,
                                         accum_out=f1ssq[:, ot:ot + 1])

            ar3_i = dr.tile([1536], F32, name="ar3_i")
            ar3_o = dr.tile([1536], F32, name="ar3_o")
            nc.gpsimd.dma_start(ar3_i[0:768].rearrange("(p t) -> p t", p=128), f1sum[:])
            nc.gpsimd.dma_start(ar3_i[768:1536].rearrange("(p t) -> p t", p=128), f1ssq[:])
            nc.gpsimd.collective_compute("AllReduce", ALU.add, replica_groups=RG,
                                         ins=[ar3_i.opt()], outs=[ar3_o.opt()])
            f1sum_g = sb.tile([128, 6], F32, name="f1sum_g")
            f1ssq_g = sb.tile([128, 6], F32, name="f1ssq_g")
            nc.gpsimd.dma_start(f1sum_g[:], ar3_o[0:768].rearrange("(p t) -> p t", p=128))
            nc.gpsimd.dma_start(f1ssq_g[:], ar3_o[768:1536].rearrange("(p t) -> p t", p=128))
            s_f1, t_f1 = _bn_scale_shift(nc, sb, f1sum_g, f1ssq_g, gf1_t, bf1_t,
                                         6, "f1")
            # g1 = gelu(h1*s + t) -> bf16 (shares the bfk slot with qkb)
            g1_t = sb.tile([128, 6 * N], BF16, name="g1_t", tag="bfk", bufs=1)
            for ot in range(6):
                nc.scalar.activation(g1_t[:, ot * N:(ot + 1) * N],
                                     h1_t[:, ot * N:(ot + 1) * N],
                                     AF.Gelu, bias=t_f1[:, ot:ot + 1],
                                     scale=s_f1[:, ot:ot + 1])

            # ---------- fc2 + BN + residual ----------
            h2_t = sb.tile([128, 3 * N], F32, name="h2_t", tag="c3k", bufs=2)
            with nc.named_scope("fc2"):
                for ot in range(3):
                    cps = ps.tile([128, N], F32, name="cps2", tag="big", bufs=2)
                    for cc in range(6):
                        lhs = wf2_t[:, cc * C + ot * 128: cc * C + ot * 128 + 128]
                        rhs = g1_t[:, cc * N:(cc + 1) * N]
                        for nh in range(2):
                            nc.tensor.matmul(cps[:, nh * 512:(nh + 1) * 512], lhs,
                                             rhs[:, nh * 512:(nh + 1) * 512],
                                             start=(cc == 0), stop=(cc == 5))
                    nc.vector.tensor_scalar(h2_t[:, ot * N:(ot + 1) * N], cps[:],
                                            1.0, None, ALU.mult, ALU.add,
                                            accum_out=f2sum[:, ot:ot + 1])
                    sqs = sb.tile([128, N], BF16, name="sqs2", tag="sqs", bufs=2)
                    nc.scalar.activation(sqs[:], cps[:], AF.Square,
                                         accum_out=f2ssq[:, ot:ot + 1])

            ar4_i = dr.tile([768], F32, name="ar4_i")
            ar4_o = dr.tile([768], F32, name="ar4_o")
            nc.gpsimd.dma_start(ar4_i[0:384].rearrange("(p t) -> p t", p=128), f2sum[:])
            nc.gpsimd.dma_start(ar4_i[384:768].rearrange("(p t) -> p t", p=128), f2ssq[:])
            nc.gpsimd.collective_compute("AllReduce", ALU.add, replica_groups=RG,
                                         ins=[ar4_i.opt()], outs=[ar4_o.opt()])
            f2sum_g = sb.tile([128, 3], F32, name="f2sum_g")
            f2ssq_g = sb.tile([128, 3], F32, name="f2ssq_g")
            nc.gpsimd.dma_start(f2sum_g[:], ar4_o[0:384].rearrange("(p t) -> p t", p=128))
            nc.gpsimd.dma_start(f2ssq_g[:], ar4_o[384:768].rearrange("(p t) -> p t", p=128))
            s_f2, t_f2 = _bn_scale_shift(nc, sb, f2sum_g, f2ssq_g, gf2_t, bf2_t,
                                         3, "f2")
            out_sb = sb.tile([128, 3 * N], F32, name="out_sb", tag="c3k", bufs=2)
            for ot in range(3):
                xs = x1_t[:, ot * N:(ot + 1) * N]
                nc.vector.tensor_scalar(xs, xs, t_f2[:, ot:ot + 1], None, ALU.add)
                nc.vector.scalar_tensor_tensor(out_sb[:, ot * N:(ot + 1) * N],
                                               h2_t[:, ot * N:(ot + 1) * N],
                                               s_f2[:, ot:ot + 1], xs,
                                               ALU.mult, ALU.add)
                nc.sync.dma_start(out_d[ot * 128:(ot + 1) * 128, :],
                                  out_sb[:, ot * N:(ot + 1) * N])

    nc.compile()
    return nc


_CACHE = {}


def _get_nc():
    if "nc" not in _CACHE:
        _CACHE["nc"] = build()
    return _CACHE["nc"]


def _pack_pm(v, k):
    """[128*k] vector -> [128, k] partition-major (col t = channels t*128..)."""
    return np.ascontiguousarray(v.reshape(k, 128).T.astype(np.float32))


def prep_inputs(x, w_qkv, g_qkv, b_qkv, w_merge, g_merge, b_merge,
                w_fc1, g_fc1, b_fc1, w_fc2, g_fc2, b_fc2):
    import ml_dtypes

    BF = ml_dtypes.bfloat16
    x = np.asarray(x, np.float32)
    w_qkv = np.asarray(w_qkv, np.float32)
    g_qkv = np.asarray(g_qkv, np.float32)
    b_qkv = np.asarray(b_qkv, np.float32)

    hidx = np.arange(HEADS)[:, None]
    q_idx = (hidx * 192 + np.arange(0, DK)[None, :]).reshape(-1)
    k_idx = (hidx * 192 + np.arange(DK, 2 * DK)[None, :]).reshape(-1)
    v_idx = (hidx * 192 + np.arange(2 * DK, 192)[None, :]).reshape(-1)

    scale = np.float32(DK ** -0.5)
    w_qk = np.concatenate([w_qkv[q_idx], w_qkv[k_idx]], axis=0)   # [512, 384]
    gqk = np.concatenate([g_qkv[q_idx] * scale, g_qkv[k_idx]])
    bqk = np.concatenate([b_qkv[q_idx] * scale, b_qkv[k_idx]])

    common = {
        "wqk": np.ascontiguousarray(w_qk.T).astype(BF),            # [384, 512]
        "wv": np.ascontiguousarray(w_qkv[v_idx].T).astype(BF),     # [384, 1024]
        "wm": np.ascontiguousarray(np.asarray(w_merge, np.float32).T).astype(BF),
        "wf1": np.ascontiguousarray(np.asarray(w_fc1, np.float32).T).astype(BF),
        "wf2": np.ascontiguousarray(np.asarray(w_fc2, np.float32).T).astype(BF),
        "gqk": _pack_pm(gqk, 4),
        "bqk": _pack_pm(bqk, 4),
        "gv": _pack_pm(g_qkv[v_idx], 8),
        "bv": _pack_pm(b_qkv[v_idx], 8),
        "gm": _pack_pm(np.asarray(g_merge, np.float32), 3),
        "bm": _pack_pm(np.asarray(b_merge, np.float32), 3),
        "gf1": _pack_pm(np.asarray(g_fc1, np.float32), 6),
        "bf1": _pack_pm(np.asarray(b_fc1, np.float32), 6),
        "gf2": _pack_pm(np.asarray(g_fc2, np.float32), 3),
        "bf2": _pack_pm(np.asarray(b_fc2, np.float32), 3),
    }
    in_maps = []
    for b in range(N_CORES):
        m = dict(common)
        m["xb"] = np.ascontiguousarray(x[b].reshape(C, N)).astype(BF)
        in_maps.append(m)
    return in_maps


def kernel(**inputs):
    nc = _get_nc()
    in_maps = prep_inputs(**inputs)
    res = run_bass_kernel_spmd(nc, in_maps, list(range(N_CORES)))
    out = np.stack([np.asarray(res.results[b]["out"], np.float32).reshape(C, 32, 32)
                    for b in range(N_CORES)])
    return out
